# revision 1
# baseline (speedup 1.0000x reference)
"""GQA attention (B=1, T=2048, D=2048, H=32, KVH=8, HD=64) on 8 TRN2 cores.

Head-tensor-parallel: core c owns kv-head c and q-heads 4c..4c+3.
wq/wk/wv column-parallel, wo row-parallel; partials summed on host.
"""
import sys

if "/opt/trn_rl_repo" not in sys.path:
    sys.path.insert(0, "/opt/trn_rl_repo")

import numpy as np
import ml_dtypes

import concourse.bacc as bacc
import concourse.mybir as mybir
import concourse.tile as tile
from concourse.bass_utils import run_bass_kernel_spmd

BF16 = ml_dtypes.bfloat16
T, D, H, KVH, HD = 2048, 2048, 32, 8, 64
NCORES = 8
HPC = H // NCORES            # 4 q heads per core
KT, PT = 16, 128             # k-tiles of 128 over D
NCH = 4                      # t chunks of 512
CH = 512

_cache = {}


def _build_nc():
    if "nc" in _cache:
        return _cache["nc"]
    fp32, bf16 = mybir.dt.float32, mybir.dt.bfloat16
    Exp = mybir.ActivationFunctionType.Exp
    mult = mybir.AluOpType.mult
    nc = bacc.Bacc("TRN2", target_bir_lowering=False, debug=False,
                   num_devices=NCORES)

    xt_d = nc.dram_tensor("xt", [D, T], bf16, kind="ExternalInput")
    wq_d = nc.dram_tensor("wq", [D, HPC * HD], bf16, kind="ExternalInput")
    wkv_d = nc.dram_tensor("wkv", [D, 2 * HD], bf16, kind="ExternalInput")
    wo_d = nc.dram_tensor("wo", [HPC * HD, D], bf16, kind="ExternalInput")
    cs4_d = nc.dram_tensor("cs4", [PT, T], bf16, kind="ExternalInput")
    sn4_d = nc.dram_tensor("sn4", [PT, T], bf16, kind="ExternalInput")
    pe_d = nc.dram_tensor("permE", [PT, 2 * PT], bf16, kind="ExternalInput")
    po_d = nc.dram_tensor("permO", [PT, 2 * PT], bf16, kind="ExternalInput")
    id_d = nc.dram_tensor("ident", [PT, PT], bf16, kind="ExternalInput")
    mk_d = nc.dram_tensor("masks", [PT, 4, NCH * CH], bf16, kind="ExternalInput")
    out_d = nc.dram_tensor("partial", [T, D], bf16, kind="ExternalOutput")

    with tile.TileContext(nc) as tc:
        with tc.tile_pool(name="const", bufs=1) as const, \
             tc.tile_pool(name="xtp", bufs=KT) as xtp, \
             tc.tile_pool(name="persist", bufs=1) as persist:

            # ---- loads: small consts first, then xt stream, then wq/wo ----
            wkv_sb = const.tile([PT, KT, 2 * HD], bf16, tag="wkv")
            nc.sync.dma_start(wkv_sb[:], wkv_d.ap().rearrange("(k p) m -> p k m", p=PT))
            cs4 = const.tile([PT, T], bf16, tag="cs4")
            nc.sync.dma_start(cs4[:], cs4_d.ap())
            sn4 = const.tile([PT, T], bf16, tag="sn4")
            nc.sync.dma_start(sn4[:], sn4_d.ap())
            permE = const.tile([PT, 2 * PT], bf16, tag="permE")
            nc.sync.dma_start(permE[:], pe_d.ap())
            permO = const.tile([PT, 2 * PT], bf16, tag="permO")
            nc.sync.dma_start(permO[:], po_d.ap())
            ident = const.tile([PT, PT], bf16, tag="ident")
            nc.sync.dma_start(ident[:], id_d.ap())
            masks = const.tile([PT, 4, NCH * CH], bf16, tag="masks")
            nc.sync.dma_start(masks[:], mk_d.ap())
            xt = []
            for k in range(KT):
                t_ = xtp.tile([PT, T], bf16, tag="xt")
                nc.sync.dma_start(t_[:], xt_d.ap()[k * PT:(k + 1) * PT, :])
                xt.append(t_)
            wq_sb = const.tile([PT, KT, HPC * HD], bf16, tag="wq")
            nc.sync.dma_start(wq_sb[:], wq_d.ap().rearrange("(k p) m -> p k m", p=PT))
            wo_sb = const.tile([PT, 2, D], bf16, tag="wo")
            nc.sync.dma_start(wo_sb[:], wo_d.ap().rearrange("(s p) m -> p s m", p=PT))
            ones_v = const.tile([1, 1], bf16, tag="ones_v")
            nc.vector.memset(ones_v[:], 1.0)

            # persistent activations: qtc[j] = [h0|h1|h2|h3] qT for chunk j
            qtc = [persist.tile([64, HPC * CH], bf16, tag=f"qtc{j}", name=f"qtc{j}")
                   for j in range(NCH)]
            kt = persist.tile([64, T], bf16, tag="kt")
            vx = [persist.tile([PT, HD + 1], bf16, tag=f"vx{s}", name=f"vx{s}")
                  for s in range(KT)]
            ot = [persist.tile([PT, T], bf16, tag=f"ot{p}", name=f"ot{p}")
                  for p in range(2)]

            # ---- phase A: kv projection (k-outer, paced to xt arrivals) ----
            with tc.tile_pool(name="kvp", bufs=1, space="PSUM") as kvp, \
                 tc.tile_pool(name="vtrp", bufs=2, space="PSUM") as vtrp, \
                 tc.tile_pool(name="tmpa", bufs=2) as tmpa:
                KV = [kvp.tile([PT, CH], fp32, tag=f"kv{j}", name=f"kv{j}")
                      for j in range(NCH)]
                for k in range(KT):
                    for j in range(NCH):
                        nc.tensor.matmul(KV[j][:], wkv_sb[:, k, :],
                                         xt[k][:, j * CH:(j + 1) * CH],
                                         start=(k == 0), stop=(k == KT - 1))
                for j in range(NCH):
                    jsl = slice(j * CH, (j + 1) * CH)
                    k1 = tmpa.tile([32, CH], fp32, tag="k1")
                    k2 = tmpa.tile([32, CH], fp32, tag="k2")
                    nc.vector.tensor_tensor(k1[:], KV[j][0:32, :], cs4[0:32, jsl], mult)
                    nc.vector.tensor_tensor(k2[:], KV[j][32:64, :], sn4[0:32, jsl], mult)
                    nc.vector.tensor_sub(kt[0:32, jsl], k1[:], k2[:])
                    k3 = tmpa.tile([32, CH], fp32, tag="k1")
                    k4 = tmpa.tile([32, CH], fp32, tag="k2")
                    nc.vector.tensor_tensor(k3[:], KV[j][0:32, :], sn4[0:32, jsl], mult)
                    nc.vector.tensor_tensor(k4[:], KV[j][32:64, :], cs4[0:32, jsl], mult)
                    nc.vector.tensor_add(kt[32:64, jsl], k3[:], k4[:])
                    vt = tmpa.tile([64, CH], bf16, tag="vt")
                    nc.vector.tensor_copy(vt[:], KV[j][64:PT, :])
                    for u in range(4):
                        s_idx = 4 * j + u
                        vtr = vtrp.tile([PT, 64], bf16, tag="vtr")
                        nc.tensor.transpose(vtr[:], vt[:, u * PT:(u + 1) * PT],
                                            ident[:64, :64])
                        nc.vector.tensor_copy(vx[s_idx][:, 0:HD], vtr[:])
                        nc.vector.memset(vx[s_idx][:, HD:HD + 1], 1.0)

            # ---- phase B: q projection + rope + repack ----
            with tc.tile_pool(name="qe", bufs=2, space="PSUM") as qep, \
                 tc.tile_pool(name="qpp", bufs=2, space="PSUM") as qpp, \
                 tc.tile_pool(name="tmpb", bufs=2) as tmpb:
                for j in range(NCH):
                    jsl = slice(j * CH, (j + 1) * CH)
                    E = qep.tile([PT, CH], fp32, tag="E")
                    O = qep.tile([PT, CH], fp32, tag="O")
                    for k in range(KT):
                        st, sp = (k == 0), (k == KT - 1)
                        nc.tensor.matmul(E[:], wq_sb[:, k, 0:PT], xt[k][:, jsl],
                                         start=st, stop=sp)
                        nc.tensor.matmul(O[:], wq_sb[:, k, PT:2 * PT], xt[k][:, jsl],
                                         start=st, stop=sp)
                    t1 = tmpb.tile([PT, CH], fp32, tag="t1")
                    t2 = tmpb.tile([PT, CH], fp32, tag="t2")
                    rE = tmpb.tile([PT, CH], bf16, tag="rE")
                    rO = tmpb.tile([PT, CH], bf16, tag="rO")
                    nc.vector.tensor_tensor(t1[:], E[:], cs4[:, jsl], mult)
                    nc.vector.tensor_tensor(t2[:], O[:], sn4[:, jsl], mult)
                    nc.vector.tensor_sub(rE[:], t1[:], t2[:])
                    t3 = tmpb.tile([PT, CH], fp32, tag="t1")
                    t4 = tmpb.tile([PT, CH], fp32, tag="t2")
                    nc.vector.tensor_tensor(t3[:], E[:], sn4[:, jsl], mult)
                    nc.vector.tensor_tensor(t4[:], O[:], cs4[:, jsl], mult)
                    nc.vector.tensor_add(rO[:], t3[:], t4[:])
                    for h in range(HPC):
                        qp = qpp.tile([64, CH], fp32, tag="qp")
                        nc.tensor.matmul(qp[:], permE[:, 64 * h:64 * h + 64],
                                         rE[:], start=True, stop=False)
                        nc.tensor.matmul(qp[:], permO[:, 64 * h:64 * h + 64],
                                         rO[:], start=False, stop=True)
                        nc.vector.tensor_copy(qtc[j][:, h * CH:(h + 1) * CH], qp[:])

            # ---- phase C: attention (4-head quad tiles per (i, j)) ----
            with tc.tile_pool(name="sc", bufs=1, space="PSUM") as scp, \
                 tc.tile_pool(name="pv", bufs=1, space="PSUM") as pvp, \
                 tc.tile_pool(name="ex", bufs=3) as exp_pool, \
                 tc.tile_pool(name="nrm", bufs=2) as nrm:
                for j in range(NCH):
                    pv = [pvp.tile([HD + 1, CH], fp32, tag=f"pv{h}", name=f"pv{h}_{j}")
                          for h in range(HPC)]
                    for i in range(4 * j + 4):
                        ktsl = kt[:, i * PT:(i + 1) * PT]
                        sc = scp.tile([PT, HPC * CH], fp32, tag="sc")
                        for h in range(HPC):
                            nc.tensor.matmul(sc[:, h * CH:(h + 1) * CH], ktsl,
                                             qtc[j][:, h * CH:(h + 1) * CH],
                                             start=True, stop=True)
                        ex = exp_pool.tile([PT, HPC * CH], bf16, tag="ex")
                        nc.scalar.activation(ex[:], sc[:], Exp, scale=0.125)
                        if i // 4 == j:
                            nc.gpsimd.tensor_tensor(ex[:], ex[:],
                                                    masks[:, i % 4, :], mult)
                        for h in range(HPC):
                            nc.tensor.matmul(pv[h][:], vx[i],
                                             ex[:, h * CH:(h + 1) * CH],
                                             start=(i == 0), stop=(i == 4 * j + 3))
                    for h in range(HPC):
                        srow = nrm.tile([1, CH], fp32, tag="srow")
                        nc.vector.tensor_copy(srow[:], pv[h][HD:HD + 1, :])
                        rrow = nrm.tile([1, CH], fp32, tag="rrow")
                        nc.vector.reciprocal_approx_fast(rrow[:], srow[:])
                        bc = nrm.tile([64, CH], fp32, tag="bc")
                        nc.gpsimd.partition_broadcast(bc[:], rrow[:])
                        nc.vector.tensor_tensor(
                            ot[h // 2][64 * (h % 2):64 * (h % 2) + 64,
                                       j * CH:(j + 1) * CH],
                            pv[h][0:HD, :], bc[:], mult)

            # ---- phase D: output projection ----
            with tc.tile_pool(name="wp", bufs=4, space="PSUM") as wpp, \
                 tc.tile_pool(name="po", bufs=4) as pop:
                n = 0
                for tt in range(KT):
                    for dd in range(NCH):
                        wp = wpp.tile([PT, CH], fp32, tag="wp")
                        for s in range(2):
                            nc.tensor.matmul(wp[:], ot[s][:, tt * PT:(tt + 1) * PT],
                                             wo_sb[:, s, dd * CH:(dd + 1) * CH],
                                             start=(s == 0), stop=(s == 1))
                        pout = pop.tile([PT, CH], bf16, tag="po")
                        if n % 2 == 0:
                            nc.scalar.copy(pout[:], wp[:])
                        else:
                            nc.vector.tensor_copy(pout[:], wp[:])
                        n += 1
                        nc.sync.dma_start(
                            out_d.ap()[tt * PT:(tt + 1) * PT, dd * CH:(dd + 1) * CH],
                            pout[:])

    nc.compile()
    _cache["nc"] = nc
    return nc


def _host_prep(x, freqs, wq, wk, wv, wo):
    x2d = np.asarray(x, np.float32)[0]                    # [T, D]
    xt = np.ascontiguousarray(x2d.T).astype(BF16)         # [D, T]
    cos = np.cos(np.asarray(freqs, np.float32))           # [T, 32]
    sin = np.sin(np.asarray(freqs, np.float32))
    cs4 = np.ascontiguousarray(np.tile(cos.T, (4, 1)))    # [128, T]
    sn4 = np.ascontiguousarray(np.tile(sin.T, (4, 1)))

    ev, od = np.arange(0, HD, 2), np.arange(1, HD, 2)

    # permE/permO [128, 256]: head h (cols 64h..64h+63): local row r<32 comes
    # from rE row 32h+r, r>=32 from rO row 32h+(r-32)
    permE = np.zeros((PT, 2 * PT), np.float32)
    permO = np.zeros((PT, 2 * PT), np.float32)
    for h in range(HPC):
        for r in range(32):
            permE[32 * h + r, 64 * h + r] = 1.0
            permO[32 * h + r, 64 * h + 32 + r] = 1.0

    ident = np.eye(PT, dtype=np.float32)

    # masks[sig, r, :] tiled x4 for the 4-head quad layout
    m1 = np.zeros((PT, 4, CH), np.float32)
    sig = np.arange(PT)[:, None]
    kap = np.arange(CH)[None, :]
    for r in range(4):
        m1[:, r, :] = (kap >= sig + PT * r).astype(np.float32)
    masks = np.ascontiguousarray(np.tile(m1, (1, 1, HPC)))  # [128, 4, 2048]

    wq_f = np.asarray(wq, np.float32)
    wk_f = np.asarray(wk, np.float32)
    wv_f = np.asarray(wv, np.float32)
    wo_f = np.asarray(wo, np.float32)

    in_maps = []
    for c in range(NCORES):
        # wq for 4 heads, evens-major-across-heads packing:
        # cols 0:128 = [h0 evens, h1 evens, h2 evens, h3 evens], 128:256 odds
        blocks = [wq_f[:, (c * HPC + h) * HD:(c * HPC + h + 1) * HD] for h in range(HPC)]
        wq_c = np.concatenate([b[:, ev] for b in blocks] + [b[:, od] for b in blocks], axis=1)
        kblk = wk_f[:, c * HD:(c + 1) * HD]
        wkv_c = np.concatenate([kblk[:, ev], kblk[:, od],
                                wv_f[:, c * HD:(c + 1) * HD]], axis=1)
        wo_c = wo_f[c * HPC * HD:(c + 1) * HPC * HD, :]
        in_maps.append({
            "xt": xt,
            "wq": np.ascontiguousarray(wq_c).astype(BF16),
            "wkv": np.ascontiguousarray(wkv_c).astype(BF16),
            "wo": np.ascontiguousarray(wo_c).astype(BF16),
            "cs4": cs4.astype(BF16),
            "sn4": sn4.astype(BF16),
            "permE": permE.astype(BF16),
            "permO": permO.astype(BF16),
            "ident": ident.astype(BF16),
            "masks": masks.astype(BF16),
        })
    return in_maps


def run(inputs, trace=False, tmpdir=None):
    nc = _build_nc()
    in_maps = _host_prep(**inputs)
    res = run_bass_kernel_spmd(nc, in_maps, list(range(NCORES)),
                               trace=trace, tmpdir=tmpdir)
    acc = np.zeros((T, D), np.float32)
    for c in range(NCORES):
        acc += res.results[c]["partial"].astype(np.float32)
    return acc[None], res


def kernel(**inputs):
    out, _ = run(inputs, trace=False)
    return out



# revision 17
# speedup vs baseline: 1.9081x; 1.9081x over previous
"""GQA attention (B=1, T=2048, D=2048, H=32, KVH=8, HD=64) on 8 TRN2 cores.

Head-tensor-parallel: core c owns kv-head c and q-heads 4c..4c+3.
wq/wk/wv column-parallel, wo row-parallel; partials summed on host.

Pipelined layout: QKV projections streamed against the xt DMA, attention
at 256-col granularity with software-pipelined scores->exp->PV and the
output projection paced in as tensor-engine filler, so the PE never
idles long enough for the HAM clock gate to re-throttle.
"""
import sys

if "/opt/trn_rl_repo" not in sys.path:
    sys.path.insert(0, "/opt/trn_rl_repo")

import os
import numpy as np
import ml_dtypes

DEBUG = os.environ.get("BASSDBG", "0") == "1"

import concourse.bacc as bacc
import concourse.mybir as mybir
import concourse.tile as tile
from concourse.bass_utils import run_bass_kernel_spmd

BF16 = ml_dtypes.bfloat16
T, D, H, KVH, HD = 2048, 2048, 32, 8, 64
NCORES = 8
HPC = H // NCORES            # 4 q heads per core
KT, PT = 16, 128             # k-tiles of 128 over D
NJ, CHB = 4, 512             # projection chunks of 512
NCC, CH = 8, 256             # attention chunks of 256
NTT = T // PT                # 16 output row-tiles

_cache = {}


def _build_nc():
    if "nc" in _cache:
        return _cache["nc"]
    fp32, bf16 = mybir.dt.float32, mybir.dt.bfloat16
    Exp = mybir.ActivationFunctionType.Exp
    mult = mybir.AluOpType.mult
    nc = bacc.Bacc("TRN2", target_bir_lowering=False, debug=False,
                   num_devices=NCORES)

    xt_d = nc.dram_tensor("xt", [D, T], bf16, kind="ExternalInput")
    wq_d = nc.dram_tensor("wq", [D, 2 * PT], bf16, kind="ExternalInput")
    wkv_d = nc.dram_tensor("wkv", [D, PT], bf16, kind="ExternalInput")
    wo_d = nc.dram_tensor("wo", [HPC * HD, D], bf16, kind="ExternalInput")
    cs4_d = nc.dram_tensor("cs4", [PT, T], bf16, kind="ExternalInput")
    sn4_d = nc.dram_tensor("sn4", [PT, T], bf16, kind="ExternalInput")
    id_d = nc.dram_tensor("ident", [PT, PT], bf16, kind="ExternalInput")
    mk_d = nc.dram_tensor("masks", [PT, 2, HPC * CH], bf16, kind="ExternalInput")
    out_d = nc.dram_tensor("partial", [T, D], bf16, kind="ExternalOutput")
    if DEBUG:
        dbg_ex_d = nc.dram_tensor("dbg_ex", [PT, 2, HPC * CH], bf16,
                                  kind="ExternalOutput")
        dbg_kt_d = nc.dram_tensor("dbg_kt", [64, T], bf16, kind="ExternalOutput")
        dbg_qt_d = nc.dram_tensor("dbg_qt", [64, HPC * CHB], bf16,
                                  kind="ExternalOutput")
        dbg_ot_d = nc.dram_tensor("dbg_ot", [PT, T], bf16, kind="ExternalOutput")

    with tile.TileContext(nc) as tc:
        with tc.tile_pool(name="const", bufs=1) as const, \
             tc.tile_pool(name="xtp", bufs=KT) as xtp, \
             tc.tile_pool(name="persist", bufs=1) as persist, \
             tc.tile_pool(name="rtmp", bufs=2) as rtmp, \
             tc.tile_pool(name="ex", bufs=3) as exp_pool, \
             tc.tile_pool(name="nrm", bufs=4) as nrm, \
             tc.tile_pool(name="po", bufs=3) as pop:

            # ---- DMA loads, ordered by first use ----
            wkv_sb = const.tile([PT, KT, PT], bf16, tag="wkv")
            nc.sync.dma_start(wkv_sb[:], wkv_d.ap().rearrange("(k p) m -> p k m", p=PT))
            wq_sb = const.tile([PT, KT, 2 * PT], bf16, tag="wq")
            nc.sync.dma_start(wq_sb[:], wq_d.ap().rearrange("(k p) m -> p k m", p=PT))
            xt = []
            for k in range(KT):
                t_ = xtp.tile([PT, T], bf16, tag="xt")
                nc.sync.dma_start(t_[:], xt_d.ap()[k * PT:(k + 1) * PT, :])
                xt.append(t_)
            cs4 = const.tile([PT, T], bf16, tag="cs4")
            nc.sync.dma_start(cs4[:], cs4_d.ap())
            sn4 = const.tile([PT, T], bf16, tag="sn4")
            nc.sync.dma_start(sn4[:], sn4_d.ap())
            ident = const.tile([PT, PT], bf16, tag="ident")
            nc.sync.dma_start(ident[:], id_d.ap())
            masks = const.tile([PT, 2, HPC * CH], bf16, tag="masks")
            nc.sync.dma_start(masks[:], mk_d.ap())
            wo_sb = const.tile([PT, 2, D], bf16, tag="wo")
            nc.sync.dma_start(wo_sb[:], wo_d.ap().rearrange("(s p) m -> p s m", p=PT))

            # ---- persistent activations ----
            qtc = [persist.tile([64, HPC * CHB], bf16, tag=f"qtc{j}", name=f"qtc{j}")
                   for j in range(NJ)]
            kt = persist.tile([64, T], bf16, tag="kt")
            vsb = [persist.tile([64, CHB], bf16, tag=f"vsb{j}", name=f"vsb{j}")
                   for j in range(NJ)]
            vx = [persist.tile([PT, HD + 1], bf16, tag=f"vx{s}", name=f"vx{s}")
                  for s in range(KT)]
            ot = [persist.tile([PT, T], bf16, tag=f"ot{p}", name=f"ot{p}")
                  for p in range(2)]

            # warm the ACT exp table during the DMA wait
            warm = rtmp.tile([1, 8], bf16, tag="warm")
            nc.vector.memset(warm[:], 0.0)
            wex = rtmp.tile([1, 8], bf16, tag="wex")
            nc.scalar.activation(wex[:], warm[:], Exp)
            for s in range(KT):
                nc.vector.memset(vx[s][:, HD:HD + 1], 1.0)

            def k_rope(j, KVj):
                jsl = slice(j * CHB, (j + 1) * CHB)
                k1 = rtmp.tile([32, CHB], fp32, tag="k1")
                k2 = rtmp.tile([32, CHB], fp32, tag="k2")
                nc.vector.tensor_tensor(k1[:], KVj[0:32, :], cs4[0:32, jsl], mult)
                nc.vector.tensor_tensor(k2[:], KVj[32:64, :], sn4[0:32, jsl], mult)
                nc.vector.tensor_sub(kt[0:32, jsl], k1[:], k2[:])
                k3 = rtmp.tile([32, CHB], fp32, tag="k1")
                k4 = rtmp.tile([32, CHB], fp32, tag="k2")
                nc.vector.tensor_tensor(k3[:], KVj[0:32, :], sn4[0:32, jsl], mult)
                nc.vector.tensor_tensor(k4[:], KVj[32:64, :], cs4[0:32, jsl], mult)
                nc.vector.tensor_add(kt[32:64, jsl], k3[:], k4[:])
                nc.vector.tensor_copy(vsb[j][:], KVj[64:PT, :])

            def q_rope(j, E, O):
                jsl = slice(j * CHB, (j + 1) * CHB)
                t1 = rtmp.tile([PT, CHB], fp32, tag="t1")
                t2 = rtmp.tile([PT, CHB], fp32, tag="t2")
                rE = rtmp.tile([PT, CHB], bf16, tag="rE")
                rO = rtmp.tile([PT, CHB], bf16, tag="rO")
                nc.vector.tensor_tensor(t1[:], E[:], cs4[:, jsl], mult)
                nc.vector.tensor_tensor(t2[:], O[:], sn4[:, jsl], mult)
                nc.vector.tensor_sub(rE[:], t1[:], t2[:])
                t3 = rtmp.tile([PT, CHB], fp32, tag="t1")
                t4 = rtmp.tile([PT, CHB], fp32, tag="t2")
                nc.vector.tensor_tensor(t3[:], E[:], sn4[:, jsl], mult)
                nc.vector.tensor_tensor(t4[:], O[:], cs4[:, jsl], mult)
                nc.vector.tensor_add(rO[:], t3[:], t4[:])
                # qtc[j] col = half*1024 + h*256 + tl, so that each 256-query
                # chunk has its 4 heads contiguous (single-MM score groups)
                for h in range(HPC):
                    for half in range(2):
                        dst = slice(half * HPC * CH + h * CH,
                                    half * HPC * CH + (h + 1) * CH)
                        src = slice(half * CH, (half + 1) * CH)
                        nc.vector.tensor_copy(qtc[j][0:32, dst],
                                              rE[32 * h:32 * h + 32, src])
                        nc.vector.tensor_copy(qtc[j][32:64, dst],
                                              rO[32 * h:32 * h + 32, src])

            # ---- stream window: KV proj + Q proj (chunks 0,1), k-paced ----
            with tc.tile_pool(name="kvp", bufs=1, space="PSUM") as kvp, \
                 tc.tile_pool(name="qea", bufs=1, space="PSUM") as qea:
                KV = [kvp.tile([PT, CHB], fp32, tag=f"kv{j}", name=f"kv{j}")
                      for j in range(NJ)]
                EO = [[qea.tile([PT, CHB], fp32, tag=f"eo{j}{e}", name=f"eo{j}{e}")
                       for e in range(2)] for j in range(2)]
                for k in range(KT):
                    st, sp = (k == 0), (k == KT - 1)
                    for j in range(NJ):
                        nc.tensor.matmul(KV[j][:], wkv_sb[:, k, :],
                                         xt[k][:, j * CHB:(j + 1) * CHB],
                                         start=st, stop=sp)
                    for j in range(2):
                        jsl = slice(j * CHB, (j + 1) * CHB)
                        nc.tensor.matmul(EO[j][0][:], wq_sb[:, k, 0:PT],
                                         xt[k][:, jsl], start=st, stop=sp)
                        nc.tensor.matmul(EO[j][1][:], wq_sb[:, k, PT:2 * PT],
                                         xt[k][:, jsl], start=st, stop=sp)
                for j in range(NJ):
                    k_rope(j, KV[j])
                for j in range(2):
                    q_rope(j, EO[j][0], EO[j][1])

            # ---- V transposes ----
            with tc.tile_pool(name="vtr", bufs=2, space="PSUM") as vtrp:
                for j in range(NJ):
                    for u in range(4):
                        s = 4 * j + u
                        vt_ = vtrp.tile([PT, HD], bf16, tag="vtr")
                        nc.tensor.transpose(vt_[:], vsb[j][:, u * PT:(u + 1) * PT],
                                            ident[0:64, 0:64])
                        nc.vector.tensor_copy(vx[s][:, 0:HD], vt_[:])

            # ---- attention pipeline with filler work ----
            units = [(cc, i) for cc in range(NCC) for i in range(2 * cc + 2)]
            ncopy = [0]

            def emit_sc(cc, i):
                j, half = cc // 2, cc % 2
                sc = _scp[0].tile([PT, HPC * CH], fp32, tag="sc")
                for hp in range(2):
                    base = half * HPC * CH + hp * 2 * CH
                    nc.tensor.matmul(sc[:, hp * 2 * CH:(hp + 1) * 2 * CH],
                                     kt[:, i * PT:(i + 1) * PT],
                                     qtc[j][:, base:base + 2 * CH],
                                     start=True, stop=True)
                ex = exp_pool.tile([PT, HPC * CH], bf16, tag="ex")
                nc.scalar.activation(ex[:], sc[:], Exp, scale=0.125)
                if i >= 2 * cc:
                    nc.vector.tensor_tensor(ex[:], ex[:],
                                            masks[:, i - 2 * cc, :], mult)
                if DEBUG and cc == 0:
                    nc.sync.dma_start(dbg_ex_d.ap()[:, i, :], ex[:])
                return ex

            def emit_pv(cc, i, ex):
                if i == 0:
                    _pv[0] = [_pvp[0].tile([HD + 1, CHB], fp32, tag=f"pv{p}",
                                           name=f"pv{p}_{cc}") for p in range(2)]
                pv = _pv[0]
                for p in range(2):
                    nc.tensor.matmul(pv[p][:, :],
                                     vx[i][:, 0:HD + 1],
                                     ex[:, p * 2 * CH:(p + 1) * 2 * CH],
                                     start=(i == 0), stop=(i == 2 * cc + 1))

            def emit_norm(cc):
                state["done"] = cc
                pv = _pv[0]
                for p in range(2):
                    srow = nrm.tile([1, CHB], fp32, tag="srow")
                    nc.vector.tensor_copy(srow[:], pv[p][HD:HD + 1, :])
                    rr = nrm.tile([1, CHB], fp32, tag="rr")
                    nc.vector.reciprocal_approx_fast(rr[:], srow[:])
                    bc = nrm.tile([64, CHB], fp32, tag="bc")
                    nc.gpsimd.partition_broadcast(bc[:], rr[:])
                    nc.vector.tensor_tensor(
                        ot[p][0:64, cc * CH:(cc + 1) * CH],
                        pv[p][0:HD, 0:CH], bc[:, 0:CH], mult)
                    nc.vector.tensor_tensor(
                        ot[p][64:PT, cc * CH:(cc + 1) * CH],
                        pv[p][0:HD, CH:2 * CH], bc[:, CH:2 * CH], mult)

            def pop_fillers(fillers, n):
                for _ in range(n):
                    if fillers and fillers[0][0] <= state["done"]:
                        fillers.pop(0)[1]()

            def run_units(lo_cc, hi_cc, fillers):
                for (cc, i) in units:
                    if not (lo_cc <= cc <= hi_cc):
                        continue
                    ex = emit_sc(cc, i)
                    if state["prev"] is not None:
                        pcc, pi, pex = state["prev"]
                        emit_pv(pcc, pi, pex)
                        if pi == 2 * pcc + 1:
                            emit_norm(pcc)
                    pop_fillers(fillers, state["per_unit"])
                    state["prev"] = (cc, i, ex)

            state = {"prev": None, "per_unit": 1, "done": -1}
            _scp = [None]
            _pvp = [None]
            _pv = [None]

            with tc.tile_pool(name="scp", bufs=2, space="PSUM") as scp, \
                 tc.tile_pool(name="pvp", bufs=1, space="PSUM") as pvp:
                _scp[0] = scp
                _pvp[0] = pvp

                # B fillers: q projection chunks 2,3 in k-slices + rope
                with tc.tile_pool(name="qeb", bufs=1, space="PSUM") as qeb:
                    bfill = []
                    EO2 = {}
                    for j in (2, 3):
                        EO2[j] = [qeb.tile([PT, CHB], fp32, tag=f"e2{e}",
                                           name=f"eo2_{j}{e}") for e in range(2)]

                        def mk_bslice(j, k0):
                            def f():
                                jsl = slice(j * CHB, (j + 1) * CHB)
                                for k in range(k0, k0 + 4):
                                    st, sp = (k == 0), (k == KT - 1)
                                    nc.tensor.matmul(EO2[j][0][:],
                                                     wq_sb[:, k, 0:PT],
                                                     xt[k][:, jsl],
                                                     start=st, stop=sp)
                                    nc.tensor.matmul(EO2[j][1][:],
                                                     wq_sb[:, k, PT:2 * PT],
                                                     xt[k][:, jsl],
                                                     start=st, stop=sp)
                            return f

                        for k0 in range(0, KT, 4):
                            bfill.append((-1, mk_bslice(j, k0)))
                        bfill.append(
                            (-1, lambda j=j: q_rope(j, EO2[j][0], EO2[j][1])))
                    run_units(0, 5, bfill)
                    pop_fillers(bfill, len(bfill))

                # D fillers: output projection + copies + DMA out
                with tc.tile_pool(name="wpp", bufs=2, space="PSUM") as wpp:
                    dfill = []
                    pouts = {}

                    def mk_dslice(tt, dd):
                        def f():
                            if dd == 0:
                                pouts[tt] = pop.tile([PT, D], bf16, tag="po",
                                                     name=f"po{tt}")
                            pout = pouts[tt]
                            wp = wpp.tile([PT, CHB], fp32, tag="wp")
                            for s in range(2):
                                nc.tensor.matmul(
                                    wp[:], ot[s][:, tt * PT:(tt + 1) * PT],
                                    wo_sb[:, s, dd * CHB:(dd + 1) * CHB],
                                    start=(s == 0), stop=(s == 1))
                            if ncopy[0] % 3 == 2:
                                nc.scalar.copy(pout[:, dd * CHB:(dd + 1) * CHB],
                                               wp[:])
                            else:
                                nc.vector.tensor_copy(
                                    pout[:, dd * CHB:(dd + 1) * CHB], wp[:])
                            ncopy[0] += 1
                            if dd == NJ - 1:
                                nc.sync.dma_start(
                                    out_d.ap()[tt * PT:(tt + 1) * PT, :],
                                    pout[:])
                        return f

                    # D(tt) may only be emitted after norm(cc=tt//2)
                    for tt in range(NTT):
                        for dd in range(NJ):
                            dfill.append((tt // 2, mk_dslice(tt, dd)))

                    state["per_unit"] = 2
                    run_units(6, 7, dfill)
                    pcc, pi, pex = state["prev"]
                    emit_pv(pcc, pi, pex)
                    emit_norm(pcc)
                    pop_fillers(dfill, len(dfill))
                    if DEBUG:
                        nc.sync.dma_start(dbg_kt_d.ap(), kt[:])
                        nc.sync.dma_start(dbg_qt_d.ap(), qtc[0][:])
                        nc.sync.dma_start(dbg_ot_d.ap(), ot[0][:])

    nc.compile()
    _cache["nc"] = nc
    return nc


def _host_prep(x, freqs, wq, wk, wv, wo):
    x2d = np.asarray(x, np.float32)[0]                    # [T, D]
    xt = np.ascontiguousarray(x2d.T).astype(BF16)         # [D, T]
    cos = np.cos(np.asarray(freqs, np.float32))           # [T, 32]
    sin = np.sin(np.asarray(freqs, np.float32))
    cs4 = np.ascontiguousarray(np.tile(cos.T, (4, 1)))    # [128, T]
    sn4 = np.ascontiguousarray(np.tile(sin.T, (4, 1)))

    ev, od = np.arange(0, HD, 2), np.arange(1, HD, 2)
    ident = np.eye(PT, dtype=np.float32)

    # diag masks for the two 128-row key tiles of a 256-col query chunk
    sig = np.arange(PT)[:, None]
    kap = np.arange(CH)[None, :]
    m = np.zeros((PT, 2, CH), np.float32)
    m[:, 0, :] = (kap >= sig).astype(np.float32)
    m[:, 1, :] = (kap >= sig + PT).astype(np.float32)
    masks = np.ascontiguousarray(np.tile(m, (1, 1, HPC)))  # [128, 2, 1024]

    wq_f = np.asarray(wq, np.float32)
    wk_f = np.asarray(wk, np.float32)
    wv_f = np.asarray(wv, np.float32)
    wo_f = np.asarray(wo, np.float32)

    in_maps = []
    for c in range(NCORES):
        blocks = [wq_f[:, (c * HPC + h) * HD:(c * HPC + h + 1) * HD]
                  for h in range(HPC)]
        wq_c = np.concatenate([b[:, ev] for b in blocks]
                              + [b[:, od] for b in blocks], axis=1)
        kblk = wk_f[:, c * HD:(c + 1) * HD]
        wkv_c = np.concatenate([kblk[:, ev], kblk[:, od],
                                wv_f[:, c * HD:(c + 1) * HD]], axis=1)
        wo_c = wo_f[c * HPC * HD:(c + 1) * HPC * HD, :]
        in_maps.append({
            "xt": xt,
            "wq": np.ascontiguousarray(wq_c).astype(BF16),
            "wkv": np.ascontiguousarray(wkv_c).astype(BF16),
            "wo": np.ascontiguousarray(wo_c).astype(BF16),
            "cs4": cs4.astype(BF16),
            "sn4": sn4.astype(BF16),
            "ident": ident.astype(BF16),
            "masks": masks.astype(BF16),
        })
    return in_maps


def run(inputs, trace=False, tmpdir=None):
    nc = _build_nc()
    in_maps = _host_prep(**inputs)
    res = run_bass_kernel_spmd(nc, in_maps, list(range(NCORES)),
                               trace=trace, tmpdir=tmpdir)
    acc = np.zeros((T, D), np.float32)
    for c in range(NCORES):
        acc += res.results[c]["partial"].astype(np.float32)
    return acc[None], res


def kernel(**inputs):
    out, _ = run(inputs, trace=False)
    return out


# revision 27
# speedup vs baseline: 1.9481x; 1.0210x over previous
"""GQA attention (B=1, T=2048, D=2048, H=32, KVH=8, HD=64) on 8 TRN2 cores.

Head-tensor-parallel: core c owns kv-head c and q-heads 4c..4c+3.
wq/wk/wv column-parallel, wo row-parallel; partials summed on host.

Pipelined layout: QKV projections streamed against the xt DMA, attention
at 256-col granularity with software-pipelined scores->exp->PV and the
output projection paced in as tensor-engine filler, so the PE never
idles long enough for the HAM clock gate to re-throttle.
"""
import sys

if "/opt/trn_rl_repo" not in sys.path:
    sys.path.insert(0, "/opt/trn_rl_repo")

import os
import numpy as np
import ml_dtypes

DEBUG = os.environ.get("BASSDBG", "0") == "1"

import concourse.bacc as bacc
import concourse.mybir as mybir
import concourse.tile as tile
from concourse.bass_utils import run_bass_kernel_spmd

BF16 = ml_dtypes.bfloat16
T, D, H, KVH, HD = 2048, 2048, 32, 8, 64
NCORES = 8
HPC = H // NCORES            # 4 q heads per core
KT, PT = 16, 128             # k-tiles of 128 over D
NJ, CHB = 4, 512             # projection chunks of 512
NCC, CH = 8, 256             # attention chunks of 256
NTT = T // PT                # 16 output row-tiles

_cache = {}


def _build_nc():
    if "nc" in _cache:
        return _cache["nc"]
    fp32, bf16 = mybir.dt.float32, mybir.dt.bfloat16
    Exp = mybir.ActivationFunctionType.Exp
    mult = mybir.AluOpType.mult
    nc = bacc.Bacc("TRN2", target_bir_lowering=False, debug=False,
                   num_devices=NCORES)

    xt_d = nc.dram_tensor("xt", [D, T], bf16, kind="ExternalInput")
    wq_d = nc.dram_tensor("wq", [D, 2 * PT], bf16, kind="ExternalInput")
    wkv_d = nc.dram_tensor("wkv", [D, PT], bf16, kind="ExternalInput")
    wo_d = nc.dram_tensor("wo", [HPC * HD, D], bf16, kind="ExternalInput")
    cs4_d = nc.dram_tensor("cs4", [PT, T], bf16, kind="ExternalInput")
    sn4_d = nc.dram_tensor("sn4", [PT, T], bf16, kind="ExternalInput")
    id_d = nc.dram_tensor("ident", [64, 64], bf16, kind="ExternalInput")
    mk_d = nc.dram_tensor("masks", [PT, 2, HPC * CH], bf16, kind="ExternalInput")
    out_d = nc.dram_tensor("partial", [T, D], bf16, kind="ExternalOutput")
    if DEBUG:
        dbg_ex_d = nc.dram_tensor("dbg_ex", [PT, 2, HPC * CH], bf16,
                                  kind="ExternalOutput")
        dbg_kt_d = nc.dram_tensor("dbg_kt", [64, T], bf16, kind="ExternalOutput")
        dbg_qt_d = nc.dram_tensor("dbg_qt", [64, HPC * CHB], bf16,
                                  kind="ExternalOutput")
        dbg_ot_d = nc.dram_tensor("dbg_ot", [PT, T], bf16, kind="ExternalOutput")

    with tile.TileContext(nc) as tc:
        with tc.tile_pool(name="const", bufs=1) as const, \
             tc.tile_pool(name="xtp", bufs=KT) as xtp, \
             tc.tile_pool(name="persist", bufs=1) as persist, \
             tc.tile_pool(name="rtmp", bufs=2) as rtmp, \
             tc.tile_pool(name="ex", bufs=3) as exp_pool, \
             tc.tile_pool(name="nrm", bufs=4) as nrm, \
             tc.tile_pool(name="po", bufs=3) as pop:

            # ---- DMA loads, ordered by first use ----
            wkv_sb = const.tile([PT, KT, PT], bf16, tag="wkv")
            nc.sync.dma_start(wkv_sb[:], wkv_d.ap().rearrange("(k p) m -> p k m", p=PT))
            wq_sb = const.tile([PT, KT, 2 * PT], bf16, tag="wq")
            nc.sync.dma_start(wq_sb[:], wq_d.ap().rearrange("(k p) m -> p k m", p=PT))
            xt = []
            for k in range(KT):
                t_ = xtp.tile([PT, T], bf16, tag="xt")
                nc.sync.dma_start(t_[:], xt_d.ap()[k * PT:(k + 1) * PT, :])
                xt.append(t_)
            cs4 = const.tile([PT, T], bf16, tag="cs4")
            nc.sync.dma_start(cs4[:], cs4_d.ap())
            sn4 = const.tile([PT, T], bf16, tag="sn4")
            nc.sync.dma_start(sn4[:], sn4_d.ap())
            ident = const.tile([64, 64], bf16, tag="ident")
            nc.sync.dma_start(ident[:], id_d.ap())
            masks = const.tile([PT, 2, HPC * CH], bf16, tag="masks")
            nc.sync.dma_start(masks[:], mk_d.ap())
            wo_sb = const.tile([PT, 2, D], bf16, tag="wo")
            nc.sync.dma_start(wo_sb[:], wo_d.ap().rearrange("(s p) m -> p s m", p=PT))

            # ---- persistent activations ----
            qtc = [persist.tile([64, HPC * CHB], bf16, tag=f"qtc{j}", name=f"qtc{j}")
                   for j in range(NJ)]
            kt = persist.tile([64, T], bf16, tag="kt")
            ksb = [persist.tile([64, CHB], bf16, tag=f"ksb{j}", name=f"ksb{j}")
                   for j in range(NJ)]
            vsb = [persist.tile([64, CHB], bf16, tag=f"vsb{j}", name=f"vsb{j}")
                   for j in range(NJ)]
            vx = [persist.tile([PT, HD + 1], bf16, tag=f"vx{s}", name=f"vx{s}")
                  for s in range(KT)]
            ot = [persist.tile([PT, T], bf16, tag=f"ot{p}", name=f"ot{p}")
                  for p in range(2)]

            # warm the ACT exp table during the DMA wait
            warm = rtmp.tile([1, 8], bf16, tag="warm")
            nc.vector.memset(warm[:], 0.0)
            wex = rtmp.tile([1, 8], bf16, tag="wex")
            nc.scalar.activation(wex[:], warm[:], Exp)
            for s in range(KT):
                nc.vector.memset(vx[s][:, HD:HD + 1], 1.0)

            def k_rope(j, eng):
                # reads ksb[j] (bf16 SBUF, 2x DVE mode)
                jsl = slice(j * CHB, (j + 1) * CHB)
                kv_ = ksb[j]
                k1 = rtmp.tile([32, CHB], bf16, tag="k1")
                k2 = rtmp.tile([32, CHB], bf16, tag="k2")
                eng.tensor_tensor(k1[:], kv_[0:32, :], cs4[0:32, jsl], mult)
                eng.tensor_tensor(k2[:], kv_[32:64, :], sn4[32:64, jsl], mult)
                eng.tensor_sub(kt[0:32, jsl], k1[:], k2[:])
                k3 = rtmp.tile([32, CHB], bf16, tag="k1")
                k4 = rtmp.tile([32, CHB], bf16, tag="k2")
                eng.tensor_tensor(k3[:], kv_[0:32, :], sn4[0:32, jsl], mult)
                eng.tensor_tensor(k4[:], kv_[32:64, :], cs4[32:64, jsl], mult)
                eng.tensor_add(kt[32:64, jsl], k3[:], k4[:])

            def q_rope(j, eng, ebt, obt):
                # reads ebt/obt (bf16 SBUF) and writes qtc[j]:
                # col = half*1024 + h*256 + tl, so each 256-query chunk has
                # its 4 heads contiguous (single-MM score groups)
                jsl = slice(j * CHB, (j + 1) * CHB)
                t1 = rtmp.tile([PT, CHB], bf16, tag="t1")
                t2 = rtmp.tile([PT, CHB], bf16, tag="t2")
                rE = rtmp.tile([PT, CHB], bf16, tag="rE")
                rO = rtmp.tile([PT, CHB], bf16, tag="rO")
                eng.tensor_tensor(t1[:], ebt[:], cs4[:, jsl], mult)
                eng.tensor_tensor(t2[:], obt[:], sn4[:, jsl], mult)
                eng.tensor_sub(rE[:], t1[:], t2[:])
                t3 = rtmp.tile([PT, CHB], bf16, tag="t1")
                t4 = rtmp.tile([PT, CHB], bf16, tag="t2")
                eng.tensor_tensor(t3[:], ebt[:], sn4[:, jsl], mult)
                eng.tensor_tensor(t4[:], obt[:], cs4[:, jsl], mult)
                eng.tensor_add(rO[:], t3[:], t4[:])
                for h in range(HPC):
                    for half in range(2):
                        dst = slice(half * HPC * CH + h * CH,
                                    half * HPC * CH + (h + 1) * CH)
                        src = slice(half * CH, (half + 1) * CH)
                        eng.tensor_copy(qtc[j][0:32, dst],
                                        rE[32 * h:32 * h + 32, src])
                        eng.tensor_copy(qtc[j][32:64, dst],
                                        rO[32 * h:32 * h + 32, src])

            # ---- stream window: KV proj + Q proj (chunks 0,1), k-paced ----
            with tc.tile_pool(name="kvp", bufs=1, space="PSUM") as kvp, \
                 tc.tile_pool(name="qea", bufs=1, space="PSUM") as qea:
                KV = [kvp.tile([PT, CHB], fp32, tag=f"kv{j}", name=f"kv{j}")
                      for j in range(NJ)]
                EO = [[qea.tile([PT, CHB], fp32, tag=f"eo{j}{e}", name=f"eo{j}{e}")
                       for e in range(2)] for j in range(2)]
                eb = {}
                ob = {}
                for k in range(KT):
                    st, sp = (k == 0), (k == KT - 1)
                    for j in range(NJ):
                        nc.tensor.matmul(KV[j][:], wkv_sb[:, k, :],
                                         xt[k][:, j * CHB:(j + 1) * CHB],
                                         start=st, stop=sp)
                    for j in range(2):
                        jsl = slice(j * CHB, (j + 1) * CHB)
                        nc.tensor.matmul(EO[j][0][:], wq_sb[:, k, 0:PT],
                                         xt[k][:, jsl], start=st, stop=sp)
                        nc.tensor.matmul(EO[j][1][:], wq_sb[:, k, PT:2 * PT],
                                         xt[k][:, jsl], start=st, stop=sp)
                # fast ACT drain of all stream PSUM to bf16 SBUF, ordered by
                # first consumer (q-rope chunk 0 first)
                nc.vector.tensor_copy(vsb[0][:], KV[0][64:PT, :])
                eb[0] = rtmp.tile([PT, CHB], bf16, tag="eb", name="eb0")
                nc.scalar.copy(eb[0][:], EO[0][0][:])
                ob[0] = rtmp.tile([PT, CHB], bf16, tag="ob", name="ob0")
                nc.scalar.copy(ob[0][:], EO[0][1][:])
                nc.scalar.copy(ksb[0][:], KV[0][0:64, :])
                eb[1] = rtmp.tile([PT, CHB], bf16, tag="eb", name="eb1")
                nc.scalar.copy(eb[1][:], EO[1][0][:])
                ob[1] = rtmp.tile([PT, CHB], bf16, tag="ob", name="ob1")
                nc.scalar.copy(ob[1][:], EO[1][1][:])
                for j in range(1, NJ):
                    nc.vector.tensor_copy(vsb[j][:], KV[j][64:PT, :])
                    nc.scalar.copy(ksb[j][:], KV[j][0:64, :])

            # rope chunk 0 starts immediately on DVE/gpsimd so attention
            # can begin while the V transposes run on the PE
            q_rope(0, nc.vector, eb[0], ob[0])
            k_rope(0, nc.gpsimd)

            # V transposes on the PE (paced by the ksb/vsb copies); vx
            # copies split between DVE (early tiles) and ACT (late tiles)
            with tc.tile_pool(name="vtr", bufs=3, space="PSUM") as vtrp:
                for s in range(KT):
                    vt_ = vtrp.tile([PT, HD], bf16, tag="vtr")
                    nc.tensor.transpose(
                        vt_[:], vsb[s // 4][:, (s % 4) * PT:(s % 4 + 1) * PT],
                        ident[:])
                    if s < 6:
                        nc.vector.tensor_copy(vx[s][:, 0:HD], vt_[:])
                    else:
                        nc.scalar.copy(vx[s][:, 0:HD], vt_[:])

            # ---- attention pipeline with filler work ----
            units = [(cc, i) for cc in range(NCC) for i in range(2 * cc + 2)]
            ncopy = [0]

            def emit_sc(cc, i):
                j, half = cc // 2, cc % 2
                sc = _scp[0].tile([PT, HPC * CH], fp32, tag="sc")
                for hp in range(2):
                    base = half * HPC * CH + hp * 2 * CH
                    nc.tensor.matmul(sc[:, hp * 2 * CH:(hp + 1) * 2 * CH],
                                     kt[:, i * PT:(i + 1) * PT],
                                     qtc[j][:, base:base + 2 * CH],
                                     start=True, stop=True)
                ex = exp_pool.tile([PT, HPC * CH], bf16, tag="ex")
                nc.scalar.activation(ex[:], sc[:], Exp, scale=0.125)
                if i >= 2 * cc:
                    nc.vector.tensor_tensor(ex[:], ex[:],
                                            masks[:, i - 2 * cc, :], mult)
                if DEBUG and cc == 0:
                    nc.sync.dma_start(dbg_ex_d.ap()[:, i, :], ex[:])
                return ex

            def emit_pv(cc, i, ex):
                if i == 0:
                    _pv[0] = [_pvp[0].tile([HD + 1, CHB], fp32, tag=f"pv{p}",
                                           name=f"pv{p}_{cc}") for p in range(2)]
                pv = _pv[0]
                for p in range(2):
                    nc.tensor.matmul(pv[p][:, :],
                                     vx[i][:, 0:HD + 1],
                                     ex[:, p * 2 * CH:(p + 1) * 2 * CH],
                                     start=(i == 0), stop=(i == 2 * cc + 1))

            def emit_norm(cc):
                state["done"] = cc
                pv = _pv[0]
                for p in range(2):
                    srow = nrm.tile([1, CHB], fp32, tag="srow")
                    nc.vector.tensor_copy(srow[:], pv[p][HD:HD + 1, :])
                    rr = nrm.tile([1, CHB], fp32, tag="rr")
                    nc.vector.reciprocal_approx_fast(rr[:], srow[:])
                    bc = nrm.tile([64, CHB], fp32, tag="bc")
                    nc.gpsimd.partition_broadcast(bc[:], rr[:])
                    nc.vector.tensor_tensor(
                        ot[p][0:64, cc * CH:(cc + 1) * CH],
                        pv[p][0:HD, 0:CH], bc[:, 0:CH], mult)
                    nc.vector.tensor_tensor(
                        ot[p][64:PT, cc * CH:(cc + 1) * CH],
                        pv[p][0:HD, CH:2 * CH], bc[:, CH:2 * CH], mult)

            def pop_fillers(fillers, n):
                for _ in range(n):
                    if fillers and fillers[0][0] <= state["done"]:
                        fillers.pop(0)[1]()

            def run_units(lo_cc, hi_cc, fillers):
                for (cc, i) in units:
                    if not (lo_cc <= cc <= hi_cc):
                        continue
                    ex = emit_sc(cc, i)
                    if state["prev"] is not None:
                        pcc, pi, pex = state["prev"]
                        emit_pv(pcc, pi, pex)
                        if pi == 2 * pcc + 1:
                            emit_norm(pcc)
                    pop_fillers(fillers, state["per_unit"])
                    state["prev"] = (cc, i, ex)

            state = {"prev": None, "per_unit": 1, "done": -1}
            _scp = [None]
            _pvp = [None]
            _pv = [None]

            with tc.tile_pool(name="scp", bufs=2, space="PSUM") as scp, \
                 tc.tile_pool(name="pvp", bufs=1, space="PSUM") as pvp:
                _scp[0] = scp
                _pvp[0] = pvp

                # B fillers: q projection chunks 2,3 in k-slices + rope
                with tc.tile_pool(name="qeb", bufs=1, space="PSUM") as qeb:
                    EO2 = {}
                    for j in (2, 3):
                        EO2[j] = [qeb.tile([PT, CHB], fp32, tag=f"e2{e}",
                                           name=f"eo2_{j}{e}") for e in range(2)]

                    def mk_bslice(j, k0):
                        def f():
                            jsl = slice(j * CHB, (j + 1) * CHB)
                            for k in range(k0, k0 + 4):
                                st, sp = (k == 0), (k == KT - 1)
                                nc.tensor.matmul(EO2[j][0][:],
                                                 wq_sb[:, k, 0:PT],
                                                 xt[k][:, jsl],
                                                 start=st, stop=sp)
                                nc.tensor.matmul(EO2[j][1][:],
                                                 wq_sb[:, k, PT:2 * PT],
                                                 xt[k][:, jsl],
                                                 start=st, stop=sp)
                        return f

                    def mk_bcopy(j):
                        def f():
                            eb[j] = rtmp.tile([PT, CHB], bf16, tag="eb",
                                              name=f"eb{j}")
                            nc.scalar.copy(eb[j][:], EO2[j][0][:])
                            ob[j] = rtmp.tile([PT, CHB], bf16, tag="ob",
                                              name=f"ob{j}")
                            nc.scalar.copy(ob[j][:], EO2[j][1][:])
                        return f

                    # pre-fillers: keep the PE busy while rope finishes
                    mk_bslice(2, 0)()
                    mk_bslice(2, 4)()
                    mk_bslice(2, 8)()

                    bfill = [
                        (-1, lambda: q_rope(1, nc.vector, eb[1], ob[1])),
                        (-1, lambda: k_rope(1, nc.gpsimd)),
                        (-1, mk_bslice(2, 12)),
                        (-1, mk_bcopy(2)),
                        (-1, lambda: q_rope(2, nc.vector, eb[2], ob[2])),
                        (-1, lambda: k_rope(2, nc.vector)),
                        (-1, lambda: k_rope(3, nc.gpsimd)),
                        (-1, mk_bslice(3, 0)),
                        (-1, mk_bslice(3, 4)),
                        (-1, mk_bslice(3, 8)),
                        (-1, mk_bslice(3, 12)),
                        (-1, mk_bcopy(3)),
                        (-1, lambda: q_rope(3, nc.vector, eb[3], ob[3])),
                    ]
                    run_units(0, 5, bfill)
                    pop_fillers(bfill, len(bfill))

                # D fillers: output projection + copies + DMA out
                with tc.tile_pool(name="wpp", bufs=2, space="PSUM") as wpp:
                    dfill = []
                    pouts = {}

                    def mk_dslice(tt, dd):
                        def f():
                            if dd == 0:
                                pouts[tt] = pop.tile([PT, D], bf16, tag="po",
                                                     name=f"po{tt}")
                            pout = pouts[tt]
                            wp = wpp.tile([PT, CHB], fp32, tag="wp")
                            for s in range(2):
                                nc.tensor.matmul(
                                    wp[:], ot[s][:, tt * PT:(tt + 1) * PT],
                                    wo_sb[:, s, dd * CHB:(dd + 1) * CHB],
                                    start=(s == 0), stop=(s == 1))
                            if ncopy[0] % 3 == 2:
                                nc.scalar.copy(pout[:, dd * CHB:(dd + 1) * CHB],
                                               wp[:])
                            else:
                                nc.vector.tensor_copy(
                                    pout[:, dd * CHB:(dd + 1) * CHB], wp[:])
                            ncopy[0] += 1
                            if dd == NJ - 1:
                                nc.sync.dma_start(
                                    out_d.ap()[tt * PT:(tt + 1) * PT, :],
                                    pout[:])
                        return f

                    # D(tt) may only be emitted after norm(cc=tt//2)
                    for tt in range(NTT):
                        for dd in range(NJ):
                            dfill.append((tt // 2, mk_dslice(tt, dd)))

                    state["per_unit"] = 2
                    run_units(6, 7, dfill)
                    pcc, pi, pex = state["prev"]
                    emit_pv(pcc, pi, pex)
                    emit_norm(pcc)
                    pop_fillers(dfill, len(dfill))
                    if DEBUG:
                        nc.sync.dma_start(dbg_kt_d.ap(), kt[:])
                        nc.sync.dma_start(dbg_qt_d.ap(), qtc[0][:])
                        nc.sync.dma_start(dbg_ot_d.ap(), ot[0][:])

    nc.compile()
    _cache["nc"] = nc
    return nc


def _host_prep(x, freqs, wq, wk, wv, wo):
    x2d = np.asarray(x, np.float32)[0]                    # [T, D]
    xt = np.ascontiguousarray(x2d.T).astype(BF16)         # [D, T]
    cos = np.cos(np.asarray(freqs, np.float32))           # [T, 32]
    sin = np.sin(np.asarray(freqs, np.float32))
    cs4 = np.ascontiguousarray(np.tile(cos.T, (4, 1)))    # [128, T]
    sn4 = np.ascontiguousarray(np.tile(sin.T, (4, 1)))

    ev, od = np.arange(0, HD, 2), np.arange(1, HD, 2)
    ident = np.eye(64, dtype=np.float32)

    # diag masks for the two 128-row key tiles of a 256-col query chunk
    sig = np.arange(PT)[:, None]
    kap = np.arange(CH)[None, :]
    m = np.zeros((PT, 2, CH), np.float32)
    m[:, 0, :] = (kap >= sig).astype(np.float32)
    m[:, 1, :] = (kap >= sig + PT).astype(np.float32)
    masks = np.ascontiguousarray(np.tile(m, (1, 1, HPC)))  # [128, 2, 1024]

    wq_f = np.asarray(wq, np.float32)
    wk_f = np.asarray(wk, np.float32)
    wv_f = np.asarray(wv, np.float32)
    wo_f = np.asarray(wo, np.float32)

    in_maps = []
    for c in range(NCORES):
        blocks = [wq_f[:, (c * HPC + h) * HD:(c * HPC + h + 1) * HD]
                  for h in range(HPC)]
        wq_c = np.concatenate([b[:, ev] for b in blocks]
                              + [b[:, od] for b in blocks], axis=1)
        kblk = wk_f[:, c * HD:(c + 1) * HD]
        wkv_c = np.concatenate([kblk[:, ev], kblk[:, od],
                                wv_f[:, c * HD:(c + 1) * HD]], axis=1)
        wo_c = wo_f[c * HPC * HD:(c + 1) * HPC * HD, :]
        in_maps.append({
            "xt": xt,
            "wq": np.ascontiguousarray(wq_c).astype(BF16),
            "wkv": np.ascontiguousarray(wkv_c).astype(BF16),
            "wo": np.ascontiguousarray(wo_c).astype(BF16),
            "cs4": cs4.astype(BF16),
            "sn4": sn4.astype(BF16),
            "ident": ident.astype(BF16),
            "masks": masks.astype(BF16),
        })
    return in_maps


def run(inputs, trace=False, tmpdir=None):
    nc = _build_nc()
    in_maps = _host_prep(**inputs)
    res = run_bass_kernel_spmd(nc, in_maps, list(range(NCORES)),
                               trace=trace, tmpdir=tmpdir)
    acc = np.zeros((T, D), np.float32)
    for c in range(NCORES):
        acc += res.results[c]["partial"].astype(np.float32)
    return acc[None], res


def kernel(**inputs):
    out, _ = run(inputs, trace=False)
    return out


# revision 28
# speedup vs baseline: 2.0686x; 1.0619x over previous
"""GQA attention (B=1, T=2048, D=2048, H=32, KVH=8, HD=64) on 8 TRN2 cores.

Head-tensor-parallel: core c owns kv-head c and q-heads 4c..4c+3.
wq/wk/wv column-parallel, wo row-parallel; partials summed on host.

Pipelined layout: QKV projections streamed against the xt DMA, attention
at 256-col granularity with software-pipelined scores->exp->PV and the
output projection paced in as tensor-engine filler, so the PE never
idles long enough for the HAM clock gate to re-throttle.
"""
import sys

if "/opt/trn_rl_repo" not in sys.path:
    sys.path.insert(0, "/opt/trn_rl_repo")

import os
import numpy as np
import ml_dtypes

DEBUG = os.environ.get("BASSDBG", "0") == "1"

import concourse.bacc as bacc
import concourse.mybir as mybir
import concourse.tile as tile
from concourse.bass_utils import run_bass_kernel_spmd

BF16 = ml_dtypes.bfloat16
T, D, H, KVH, HD = 2048, 2048, 32, 8, 64
NCORES = 8
HPC = H // NCORES            # 4 q heads per core
KT, PT = 16, 128             # k-tiles of 128 over D
NJ, CHB = 4, 512             # projection chunks of 512
NCC, CH = 8, 256             # attention chunks of 256
NTT = T // PT                # 16 output row-tiles

_cache = {}


def _build_nc():
    if "nc" in _cache:
        return _cache["nc"]
    fp32, bf16 = mybir.dt.float32, mybir.dt.bfloat16
    Exp = mybir.ActivationFunctionType.Exp
    mult = mybir.AluOpType.mult
    nc = bacc.Bacc("TRN2", target_bir_lowering=False, debug=False,
                   num_devices=NCORES)

    xt_d = nc.dram_tensor("xt", [D, T], bf16, kind="ExternalInput")
    wq_d = nc.dram_tensor("wq", [D, 2 * PT], bf16, kind="ExternalInput")
    wkv_d = nc.dram_tensor("wkv", [D, PT], bf16, kind="ExternalInput")
    wo_d = nc.dram_tensor("wo", [HPC * HD, D], bf16, kind="ExternalInput")
    cs4_d = nc.dram_tensor("cs4", [PT, T], bf16, kind="ExternalInput")
    sn4_d = nc.dram_tensor("sn4", [PT, T], bf16, kind="ExternalInput")
    id_d = nc.dram_tensor("ident", [64, 64], bf16, kind="ExternalInput")
    mk_d = nc.dram_tensor("masks", [PT, 2, HPC * CH], bf16, kind="ExternalInput")
    out_d = nc.dram_tensor("partial", [T, D], bf16, kind="ExternalOutput")
    if DEBUG:
        dbg_ex_d = nc.dram_tensor("dbg_ex", [PT, 2, HPC * CH], bf16,
                                  kind="ExternalOutput")
        dbg_kt_d = nc.dram_tensor("dbg_kt", [64, T], bf16, kind="ExternalOutput")
        dbg_qt_d = nc.dram_tensor("dbg_qt", [64, HPC * CHB], bf16,
                                  kind="ExternalOutput")
        dbg_ot_d = nc.dram_tensor("dbg_ot", [PT, T], bf16, kind="ExternalOutput")

    with tile.TileContext(nc) as tc:
        with tc.tile_pool(name="const", bufs=1) as const, \
             tc.tile_pool(name="xtp", bufs=KT) as xtp, \
             tc.tile_pool(name="persist", bufs=1) as persist, \
             tc.tile_pool(name="rtmp", bufs=2) as rtmp, \
             tc.tile_pool(name="ex", bufs=3) as exp_pool, \
             tc.tile_pool(name="nrm", bufs=4) as nrm, \
             tc.tile_pool(name="po", bufs=3) as pop:

            # ---- DMA loads, ordered by first use ----
            wkv_sb = const.tile([PT, KT, PT], bf16, tag="wkv")
            nc.sync.dma_start(wkv_sb[:], wkv_d.ap().rearrange("(k p) m -> p k m", p=PT))
            wq_sb = const.tile([PT, KT, 2 * PT], bf16, tag="wq")
            nc.sync.dma_start(wq_sb[:], wq_d.ap().rearrange("(k p) m -> p k m", p=PT))
            xt = []
            for k in range(KT):
                t_ = xtp.tile([PT, T], bf16, tag="xt")
                nc.sync.dma_start(t_[:], xt_d.ap()[k * PT:(k + 1) * PT, :])
                xt.append(t_)
            cs4 = const.tile([PT, T], bf16, tag="cs4")
            nc.sync.dma_start(cs4[:], cs4_d.ap())
            sn4 = const.tile([PT, T], bf16, tag="sn4")
            nc.sync.dma_start(sn4[:], sn4_d.ap())
            ident = const.tile([64, 64], bf16, tag="ident")
            nc.sync.dma_start(ident[:], id_d.ap())
            masks = const.tile([PT, 2, HPC * CH], bf16, tag="masks")
            nc.sync.dma_start(masks[:], mk_d.ap())
            wo_sb = const.tile([PT, 2, D], bf16, tag="wo")
            nc.sync.dma_start(wo_sb[:], wo_d.ap().rearrange("(s p) m -> p s m", p=PT))

            # ---- persistent activations ----
            qtc = [persist.tile([64, HPC * CHB], bf16, tag=f"qtc{j}", name=f"qtc{j}")
                   for j in range(NJ)]
            kt = persist.tile([64, T], bf16, tag="kt")
            ksb = [persist.tile([64, CHB], bf16, tag=f"ksb{j}", name=f"ksb{j}")
                   for j in range(NJ)]
            vsb = [persist.tile([64, CHB], bf16, tag=f"vsb{j}", name=f"vsb{j}")
                   for j in range(NJ)]
            vx = [persist.tile([PT, HD + 1], bf16, tag=f"vx{s}", name=f"vx{s}")
                  for s in range(KT)]
            ot = [persist.tile([PT, T], bf16, tag=f"ot{p}", name=f"ot{p}")
                  for p in range(2)]

            # warm the ACT exp table during the DMA wait
            warm = rtmp.tile([1, 8], bf16, tag="warm")
            nc.vector.memset(warm[:], 0.0)
            wex = rtmp.tile([1, 8], bf16, tag="wex")
            nc.scalar.activation(wex[:], warm[:], Exp)
            for s in range(KT):
                nc.vector.memset(vx[s][:, HD:HD + 1], 1.0)

            def k_rope(j, eng):
                # reads ksb[j] (bf16 SBUF, 2x DVE mode)
                jsl = slice(j * CHB, (j + 1) * CHB)
                kv_ = ksb[j]
                k1 = rtmp.tile([32, CHB], bf16, tag="k1")
                k2 = rtmp.tile([32, CHB], bf16, tag="k2")
                eng.tensor_tensor(k1[:], kv_[0:32, :], cs4[0:32, jsl], mult)
                eng.tensor_tensor(k2[:], kv_[32:64, :], sn4[32:64, jsl], mult)
                eng.tensor_sub(kt[0:32, jsl], k1[:], k2[:])
                k3 = rtmp.tile([32, CHB], bf16, tag="k1")
                k4 = rtmp.tile([32, CHB], bf16, tag="k2")
                eng.tensor_tensor(k3[:], kv_[0:32, :], sn4[0:32, jsl], mult)
                eng.tensor_tensor(k4[:], kv_[32:64, :], cs4[32:64, jsl], mult)
                eng.tensor_add(kt[32:64, jsl], k3[:], k4[:])

            def q_rope(j, eng, ebt, obt):
                # reads ebt/obt (bf16 SBUF) and writes qtc[j]:
                # col = half*1024 + h*256 + tl, so each 256-query chunk has
                # its 4 heads contiguous (single-MM score groups)
                jsl = slice(j * CHB, (j + 1) * CHB)
                t1 = rtmp.tile([PT, CHB], bf16, tag="t1")
                t2 = rtmp.tile([PT, CHB], bf16, tag="t2")
                rE = rtmp.tile([PT, CHB], bf16, tag="rE")
                rO = rtmp.tile([PT, CHB], bf16, tag="rO")
                eng.tensor_tensor(t1[:], ebt[:], cs4[:, jsl], mult)
                eng.tensor_tensor(t2[:], obt[:], sn4[:, jsl], mult)
                eng.tensor_sub(rE[:], t1[:], t2[:])
                t3 = rtmp.tile([PT, CHB], bf16, tag="t1")
                t4 = rtmp.tile([PT, CHB], bf16, tag="t2")
                eng.tensor_tensor(t3[:], ebt[:], sn4[:, jsl], mult)
                eng.tensor_tensor(t4[:], obt[:], cs4[:, jsl], mult)
                eng.tensor_add(rO[:], t3[:], t4[:])
                for h in range(HPC):
                    for half in range(2):
                        dst = slice(half * HPC * CH + h * CH,
                                    half * HPC * CH + (h + 1) * CH)
                        src = slice(half * CH, (half + 1) * CH)
                        eng.tensor_copy(qtc[j][0:32, dst],
                                        rE[32 * h:32 * h + 32, src])
                        eng.tensor_copy(qtc[j][32:64, dst],
                                        rO[32 * h:32 * h + 32, src])

            # ---- stream window: KV proj + Q proj (chunks 0,1), k-paced ----
            with tc.tile_pool(name="kvp", bufs=1, space="PSUM") as kvp, \
                 tc.tile_pool(name="qea", bufs=1, space="PSUM") as qea:
                KV = [kvp.tile([PT, CHB], fp32, tag=f"kv{j}", name=f"kv{j}")
                      for j in range(NJ)]
                EO = [[qea.tile([PT, CHB], fp32, tag=f"eo{j}{e}", name=f"eo{j}{e}")
                       for e in range(2)] for j in range(2)]
                eb = {}
                ob = {}
                for k in range(KT):
                    st, sp = (k == 0), (k == KT - 1)
                    for j in range(NJ):
                        nc.tensor.matmul(KV[j][:], wkv_sb[:, k, :],
                                         xt[k][:, j * CHB:(j + 1) * CHB],
                                         start=st, stop=sp)
                    for j in range(2):
                        jsl = slice(j * CHB, (j + 1) * CHB)
                        nc.tensor.matmul(EO[j][0][:], wq_sb[:, k, 0:PT],
                                         xt[k][:, jsl], start=st, stop=sp)
                        nc.tensor.matmul(EO[j][1][:], wq_sb[:, k, PT:2 * PT],
                                         xt[k][:, jsl], start=st, stop=sp)
                # fast ACT drain of all stream PSUM to bf16 SBUF, ordered by
                # first consumer (q-rope chunk 0 first)
                eb[0] = rtmp.tile([PT, CHB], bf16, tag="eb", name="eb0")
                nc.scalar.copy(eb[0][:], EO[0][0][:])
                ob[0] = rtmp.tile([PT, CHB], bf16, tag="ob", name="ob0")
                nc.scalar.copy(ob[0][:], EO[0][1][:])
                nc.scalar.copy(ksb[0][:], KV[0][0:64, :])
                eb[1] = rtmp.tile([PT, CHB], bf16, tag="eb", name="eb1")
                nc.scalar.copy(eb[1][:], EO[1][0][:])
                ob[1] = rtmp.tile([PT, CHB], bf16, tag="ob", name="ob1")
                nc.scalar.copy(ob[1][:], EO[1][1][:])
                for j in range(1, NJ):
                    nc.scalar.copy(ksb[j][:], KV[j][0:64, :])
                for j in range(NJ):
                    nc.vector.tensor_copy(vsb[j][:], KV[j][64:PT, :])

            # rope chunk 0 on DVE so attention can begin; all K-ropes on
            # gpsimd (their deadlines are late)
            q_rope(0, nc.vector, eb[0], ob[0])
            for j in range(NJ):
                k_rope(j, nc.gpsimd)

            # V transposes on the PE (paced by the ksb/vsb copies); vx
            # copies split between DVE (early tiles) and ACT (late tiles)
            with tc.tile_pool(name="vtr", bufs=8, space="PSUM") as vtrp:
                for s in range(KT):
                    vt_ = vtrp.tile([PT, HD], bf16, tag="vtr")
                    nc.tensor.transpose(
                        vt_[:], vsb[s // 4][:, (s % 4) * PT:(s % 4 + 1) * PT],
                        ident[:])
                    if s < 8:
                        nc.vector.tensor_copy(vx[s][:, 0:HD], vt_[:])
                    else:
                        nc.scalar.copy(vx[s][:, 0:HD], vt_[:])

            # ---- attention pipeline with filler work ----
            units = [(cc, i) for cc in range(NCC) for i in range(2 * cc + 2)]
            ncopy = [0]

            def emit_sc(cc, i):
                j, half = cc // 2, cc % 2
                sc = _scp[0].tile([PT, HPC * CH], fp32, tag="sc")
                for hp in range(2):
                    base = half * HPC * CH + hp * 2 * CH
                    nc.tensor.matmul(sc[:, hp * 2 * CH:(hp + 1) * 2 * CH],
                                     kt[:, i * PT:(i + 1) * PT],
                                     qtc[j][:, base:base + 2 * CH],
                                     start=True, stop=True)
                ex = exp_pool.tile([PT, HPC * CH], bf16, tag="ex")
                nc.scalar.activation(ex[:], sc[:], Exp, scale=0.125)
                if i >= 2 * cc:
                    nc.vector.tensor_tensor(ex[:], ex[:],
                                            masks[:, i - 2 * cc, :], mult)
                if DEBUG and cc == 0:
                    nc.sync.dma_start(dbg_ex_d.ap()[:, i, :], ex[:])
                return ex

            def emit_pv(cc, i, ex):
                if i == 0:
                    _pv[0] = [_pvp[0].tile([HD + 1, CHB], fp32, tag=f"pv{p}",
                                           name=f"pv{p}_{cc}") for p in range(2)]
                pv = _pv[0]
                for p in range(2):
                    nc.tensor.matmul(pv[p][:, :],
                                     vx[i][:, 0:HD + 1],
                                     ex[:, p * 2 * CH:(p + 1) * 2 * CH],
                                     start=(i == 0), stop=(i == 2 * cc + 1))

            def emit_norm(cc):
                state["done"] = cc
                pv = _pv[0]
                for p in range(2):
                    srow = nrm.tile([1, CHB], fp32, tag="srow")
                    nc.vector.tensor_copy(srow[:], pv[p][HD:HD + 1, :])
                    rr = nrm.tile([1, CHB], fp32, tag="rr")
                    nc.vector.reciprocal_approx_fast(rr[:], srow[:])
                    bc = nrm.tile([64, CHB], fp32, tag="bc")
                    nc.gpsimd.partition_broadcast(bc[:], rr[:])
                    nc.vector.tensor_tensor(
                        ot[p][0:64, cc * CH:(cc + 1) * CH],
                        pv[p][0:HD, 0:CH], bc[:, 0:CH], mult)
                    nc.vector.tensor_tensor(
                        ot[p][64:PT, cc * CH:(cc + 1) * CH],
                        pv[p][0:HD, CH:2 * CH], bc[:, CH:2 * CH], mult)

            def pop_fillers(fillers, n):
                for _ in range(n):
                    if fillers and fillers[0][0] <= state["done"]:
                        fillers.pop(0)[1]()

            def run_units(lo_cc, hi_cc, fillers):
                for (cc, i) in units:
                    if not (lo_cc <= cc <= hi_cc):
                        continue
                    ex = emit_sc(cc, i)
                    if state["prev"] is not None:
                        pcc, pi, pex = state["prev"]
                        emit_pv(pcc, pi, pex)
                        if pi == 2 * pcc + 1:
                            emit_norm(pcc)
                    pop_fillers(fillers, state["per_unit"])
                    state["prev"] = (cc, i, ex)

            state = {"prev": None, "per_unit": 1, "done": -1}
            _scp = [None]
            _pvp = [None]
            _pv = [None]

            with tc.tile_pool(name="scp", bufs=2, space="PSUM") as scp, \
                 tc.tile_pool(name="pvp", bufs=1, space="PSUM") as pvp:
                _scp[0] = scp
                _pvp[0] = pvp

                # B fillers: q projection chunks 2,3 in k-slices + rope
                with tc.tile_pool(name="qeb", bufs=1, space="PSUM") as qeb:
                    EO2 = {}
                    for j in (2, 3):
                        EO2[j] = [qeb.tile([PT, CHB], fp32, tag=f"e2{e}",
                                           name=f"eo2_{j}{e}") for e in range(2)]

                    def mk_bslice(j, k0):
                        def f():
                            jsl = slice(j * CHB, (j + 1) * CHB)
                            for k in range(k0, k0 + 4):
                                st, sp = (k == 0), (k == KT - 1)
                                nc.tensor.matmul(EO2[j][0][:],
                                                 wq_sb[:, k, 0:PT],
                                                 xt[k][:, jsl],
                                                 start=st, stop=sp)
                                nc.tensor.matmul(EO2[j][1][:],
                                                 wq_sb[:, k, PT:2 * PT],
                                                 xt[k][:, jsl],
                                                 start=st, stop=sp)
                        return f

                    def mk_bcopy(j):
                        def f():
                            eb[j] = rtmp.tile([PT, CHB], bf16, tag="eb",
                                              name=f"eb{j}")
                            nc.scalar.copy(eb[j][:], EO2[j][0][:])
                            ob[j] = rtmp.tile([PT, CHB], bf16, tag="ob",
                                              name=f"ob{j}")
                            nc.scalar.copy(ob[j][:], EO2[j][1][:])
                        return f

                    # pre-fillers: keep the PE busy while rope finishes
                    mk_bslice(2, 0)()
                    mk_bslice(2, 4)()
                    mk_bslice(2, 8)()

                    bfill = [
                        (-1, lambda: q_rope(1, nc.vector, eb[1], ob[1])),
                        (-1, mk_bslice(2, 12)),
                        (-1, mk_bcopy(2)),
                        (-1, lambda: q_rope(2, nc.vector, eb[2], ob[2])),
                        (-1, mk_bslice(3, 0)),
                        (-1, mk_bslice(3, 4)),
                        (-1, mk_bslice(3, 8)),
                        (-1, mk_bslice(3, 12)),
                        (-1, mk_bcopy(3)),
                        (-1, lambda: q_rope(3, nc.vector, eb[3], ob[3])),
                    ]
                    run_units(0, 5, bfill)
                    pop_fillers(bfill, len(bfill))

                # D fillers: output projection + copies + DMA out
                with tc.tile_pool(name="wpp", bufs=2, space="PSUM") as wpp:
                    dfill = []
                    pouts = {}

                    def mk_dslice(tt, dd):
                        def f():
                            if dd == 0:
                                pouts[tt] = pop.tile([PT, D], bf16, tag="po",
                                                     name=f"po{tt}")
                            pout = pouts[tt]
                            wp = wpp.tile([PT, CHB], fp32, tag="wp")
                            for s in range(2):
                                nc.tensor.matmul(
                                    wp[:], ot[s][:, tt * PT:(tt + 1) * PT],
                                    wo_sb[:, s, dd * CHB:(dd + 1) * CHB],
                                    start=(s == 0), stop=(s == 1))
                            if ncopy[0] % 3 == 2:
                                nc.scalar.copy(pout[:, dd * CHB:(dd + 1) * CHB],
                                               wp[:])
                            else:
                                nc.vector.tensor_copy(
                                    pout[:, dd * CHB:(dd + 1) * CHB], wp[:])
                            ncopy[0] += 1
                            if dd == NJ - 1:
                                nc.sync.dma_start(
                                    out_d.ap()[tt * PT:(tt + 1) * PT, :],
                                    pout[:])
                        return f

                    # D(tt) may only be emitted after norm(cc=tt//2)
                    for tt in range(NTT):
                        for dd in range(NJ):
                            dfill.append((tt // 2, mk_dslice(tt, dd)))

                    state["per_unit"] = 2
                    run_units(6, 7, dfill)
                    pcc, pi, pex = state["prev"]
                    emit_pv(pcc, pi, pex)
                    emit_norm(pcc)
                    pop_fillers(dfill, len(dfill))
                    if DEBUG:
                        nc.sync.dma_start(dbg_kt_d.ap(), kt[:])
                        nc.sync.dma_start(dbg_qt_d.ap(), qtc[0][:])
                        nc.sync.dma_start(dbg_ot_d.ap(), ot[0][:])

    nc.compile()
    _cache["nc"] = nc
    return nc


def _host_prep(x, freqs, wq, wk, wv, wo):
    x2d = np.asarray(x, np.float32)[0]                    # [T, D]
    xt = np.ascontiguousarray(x2d.T).astype(BF16)         # [D, T]
    cos = np.cos(np.asarray(freqs, np.float32))           # [T, 32]
    sin = np.sin(np.asarray(freqs, np.float32))
    cs4 = np.ascontiguousarray(np.tile(cos.T, (4, 1)))    # [128, T]
    sn4 = np.ascontiguousarray(np.tile(sin.T, (4, 1)))

    ev, od = np.arange(0, HD, 2), np.arange(1, HD, 2)
    ident = np.eye(64, dtype=np.float32)

    # diag masks for the two 128-row key tiles of a 256-col query chunk
    sig = np.arange(PT)[:, None]
    kap = np.arange(CH)[None, :]
    m = np.zeros((PT, 2, CH), np.float32)
    m[:, 0, :] = (kap >= sig).astype(np.float32)
    m[:, 1, :] = (kap >= sig + PT).astype(np.float32)
    masks = np.ascontiguousarray(np.tile(m, (1, 1, HPC)))  # [128, 2, 1024]

    wq_f = np.asarray(wq, np.float32)
    wk_f = np.asarray(wk, np.float32)
    wv_f = np.asarray(wv, np.float32)
    wo_f = np.asarray(wo, np.float32)

    in_maps = []
    for c in range(NCORES):
        blocks = [wq_f[:, (c * HPC + h) * HD:(c * HPC + h + 1) * HD]
                  for h in range(HPC)]
        wq_c = np.concatenate([b[:, ev] for b in blocks]
                              + [b[:, od] for b in blocks], axis=1)
        kblk = wk_f[:, c * HD:(c + 1) * HD]
        wkv_c = np.concatenate([kblk[:, ev], kblk[:, od],
                                wv_f[:, c * HD:(c + 1) * HD]], axis=1)
        wo_c = wo_f[c * HPC * HD:(c + 1) * HPC * HD, :]
        in_maps.append({
            "xt": xt,
            "wq": np.ascontiguousarray(wq_c).astype(BF16),
            "wkv": np.ascontiguousarray(wkv_c).astype(BF16),
            "wo": np.ascontiguousarray(wo_c).astype(BF16),
            "cs4": cs4.astype(BF16),
            "sn4": sn4.astype(BF16),
            "ident": ident.astype(BF16),
            "masks": masks.astype(BF16),
        })
    return in_maps


def run(inputs, trace=False, tmpdir=None):
    nc = _build_nc()
    in_maps = _host_prep(**inputs)
    res = run_bass_kernel_spmd(nc, in_maps, list(range(NCORES)),
                               trace=trace, tmpdir=tmpdir)
    acc = np.zeros((T, D), np.float32)
    for c in range(NCORES):
        acc += res.results[c]["partial"].astype(np.float32)
    return acc[None], res


def kernel(**inputs):
    out, _ = run(inputs, trace=False)
    return out
